# revision 48
# baseline (speedup 1.0000x reference)
"""ConvergedInhibition TRN2 kernel (fp8 DoubleRow, 2-chunk band).

The reference computes, per pixel, an FFT deconvolution along the channel
axis: y = ifft(fft(x)/fft(k)).real = circulant matmul with g = ifft(1/fft(k)).
g is a near-delta: one big tap a0 = 1.055 at offset 288 plus a small,
essentially ONE-SIDED tail h (offsets +1..+224; ||h||/||g|| = 0.13). Split
y = a0*shift(x) + h(*)x: the device computes only the correction z = W @ x
in fp8 end to end; the host adds the identity part from the fp32 input it
already holds. Because h is one-sided and ~224 wide, a rotated output
ordering z[r] = c[(r + S) % C] (S = 32) makes every output chunk zc depend
on only TWO input chunks {zc+1, zc+2} (mod 4): one K=256
perf_mode=DoubleRow fp8 matmul (2 weights/PE cell) per [128, 448] psum
tile. Chunks sit in permuted SBUF slots (a C4-cycle embedding, SLOT) so
every pair is a stride-1/2 rhs view, and gt blocks are packed so every
weight-pair stride is <= 384B - both larger-stride forms stream at half
rate on HW. ~27us of cast-paced stream; fp8 I/O
is 12.9 MB/core, so the kernel runs saturated at the per-NC HBM roofline
(~300-350 GB/s effective) nearly end to end.

Measured error 6.0e-3 vs the 2e-2 gate (fp8 quantization of x, W, z plus
the dropped-lag window; see _make_gt). Measured HW exec ~50-56us vs the
89us fp16 baseline.

Schedule highlights (per core, 4 images, raw bacc semaphores):
  - clears block: both HWDGE engines (sync/SP + scalar/ACT) clear their
    own DMA-driven semaphores and immediately launch gt + img0's first
    3 pixel-groups in parallel, overlapping the ~7us NEFF preamble.
  - loads: one [128, 3136] DMA per (img, chunk) - 128 descriptors with
    3136B lines; all four a_sb buffers are resident so no load gating.
  - tensor: 6 HAM-warmup matmuls on garbage SBUF cover the clock-gate
    window; psum banks are picked by (zc, group parity) so the cast
    reuse distance is 2 groups and casts never stall the PE.
  - casts: one combined [p, 2 zc, 448] fp32->fp8 copy per group per
    engine (DVE: zc 0,1; ACT: zc 2,3) - fits under the 1.01us matmul
    group.
  - stores: zc 0,1 on the sync HWDGE ring, zc 2,3 on the gpsimd SWDGE
    ring (separate semaphores - a HWDGE and SWDGE may not share one),
    piece-granular behind the casts, finer for the last image to cut the
    drain tail; main block skips the expensive gpsimd dge_drain.
"""

import numpy as np
import ml_dtypes

import concourse.bass as bass  # noqa: F401  (registers bass types)
import concourse.mybir as mybir
from concourse import bacc
from concourse.bass_utils import run_bass_kernel_spmd

N_CORES = 8
N, C, H, W = 32, 512, 56, 56
HW = H * W                      # 3136
IMGS = N // N_CORES             # 4 images per core
P = 128                         # partitions
NCHUNK = C // P                 # 4
PT = 448                        # pixel tile (free dim), 3136 = 7*448
NPT = HW // PT                  # 7 groups per image
ROT = 288                       # position of g's dominant (identity) tap
S_ROT = 32                      # output rotation aligning h's band to chunks
SCALE = 16.0                    # folded into W so z uses e4m3's sweet spot
IO_DT = mybir.dt.float8e4
IO_NP = ml_dtypes.float8_e4m3   # == mybir.dt.np(float8e4)

_CACHE = {}

GROUPS = IMGS * NPT             # 32 (img, cb) groups, 4 zc tiles each

# kept input chunks per output chunk: {zc+1, zc+2} mod 4. Chunks are
# stored in SBUF slot SLOT[c] (a C4-cycle embedding), which makes every
# kept pair a stride-1 or stride-2 slot view - full-rate DoubleRow rhs
# APs for all zc (stride-3 APs stream at half rate on HW).
SLOT = {0: 0, 1: 2, 2: 3, 3: 1}
# rhs slot slices (start, stop, step) per zc; slot order = chunk order
# (zc0: c1,c2 | zc1: c3,c2 | zc2: c0,c3 | zc3: c0,c1)
RHS_SL = {0: (2, 4, 1), 1: (1, 4, 2), 2: (0, 2, 1), 3: (0, 3, 2)}


def _build_nc():
    nc = bacc.Bacc("TRN2", target_bir_lowering=False, debug=False,
                   num_devices=N_CORES)
    act = nc.dram_tensor("act", [IMGS, C, HW], IO_DT, kind="ExternalInput")
    gt = nc.dram_tensor("gt", [C, C // 2], IO_DT, kind="ExternalInput")
    out = nc.dram_tensor("out", [IMGS, C, HW], IO_DT, kind="ExternalOutput")

    act_v2 = act.ap().rearrange("n (jc p) m -> n jc p m", p=P)
    gt_v = gt.ap().rearrange("(jc p) r -> p jc r", p=P)
    out_v = out.ap().rearrange("n (zc p) m -> n zc p m", p=P)

    from contextlib import ExitStack
    with ExitStack() as ctx:
        a_sb = [ctx.enter_context(
            nc.sbuf_tensor(f"a_sb{h}", [P, NCHUNK * HW], IO_DT)).ap()
            for h in range(IMGS)]
        gt_sb = ctx.enter_context(
            nc.sbuf_tensor("gt_sb", [P, NCHUNK * C // 2], IO_DT)).ap()
        o_sb = [ctx.enter_context(
            nc.sbuf_tensor(f"o_sb{i}", [P, NCHUNK * HW], IO_DT)).ap()
            for i in range(IMGS)]
        # psum: 4 banks per cast engine; [zc-pair, parity] quadrants
        ps_v = ctx.enter_context(
            nc.psum_tensor("ps_v", [P, 2048], mybir.dt.float32)).ap()
        ps_s = ctx.enter_context(
            nc.psum_tensor("ps_s", [P, 2048], mybir.dt.float32)).ap()

        a3 = [a.rearrange("p (jc m) -> p jc m", jc=NCHUNK) for a in a_sb]
        gtb = gt_sb.rearrange("p (b m) -> p b m", m=P)  # 8 128-col blocks
        # weight DR pairs [p, 2, 128], block order matching RHS_SL chunk
        # order (see _make_gt packing): all ascending block strides
        GT_W = {0: gtb[:, 3:7:3], 1: gtb[:, 5:8:2],
                2: gtb[:, 1:5:3], 3: gtb[:, 0:3:2]}
        # cast views: [p, zc-of-engine, parity, 512]
        psv4 = ps_v.rearrange("p (zc par m) -> p zc par m", zc=2, par=2)
        pss4 = ps_s.rearrange("p (zc par m) -> p zc par m", zc=2, par=2)
        o4 = [o.rearrange("p (zc m) -> p zc m", zc=NCHUNK) for o in o_sb]

        def mm_out(zc, par):
            ps = psv4 if zc < 2 else pss4
            return ps[:, zc % 2, par, :PT]

        s_gt = nc.alloc_semaphore("s_gt")
        s_ld = [nc.alloc_semaphore(f"s_ld{i}") for i in range(IMGS)]
        s_ld0h = [nc.alloc_semaphore(f"s_ld0h{h}") for h in range(2)]
        s_mm = nc.alloc_semaphore("s_mm")
        s_cast_v = nc.alloc_semaphore("s_cast_v")
        s_cast_s = nc.alloc_semaphore("s_cast_s")
        s_st = nc.alloc_semaphore("s_st")     # HWDGE (sync) stores
        s_stg = nc.alloc_semaphore("s_stg")   # SWDGE (gpsimd) stores

        # img0 prefetch pieces, aligned to PT groups: [0,4) and [4,7)
        P0 = (0, 4 * PT, HW)

        def emit_load(sync, img, jc):
            # [128, 3136] per (img, chunk): 128 descriptors, 3136B lines
            sync.dma_start(
                a3[img][:, SLOT[jc]], act_v2[img, jc],
            ).then_inc(s_ld[img], 16)

        # store pieces: (img, zc, piece-range); imgs 0-2 in halves, the
        # last image in quarters so the post-compute drain is short
        def store_pieces(img):
            bounds = (0, 2, 4, 7) if img < IMGS - 1 else (0, 2, 4, 6, 7)
            for pc in range(len(bounds) - 1):
                yield (bounds[pc] * PT, bounds[pc + 1] * PT,
                       img * NPT + bounds[pc + 1])

        # Stage 0: clear semaphores (not zeroed on alloc; DMA-driven sems
        # are cleared by the engine that issues their DMAs, so the clear
        # always precedes the first inc), then launch gt + img0 from BOTH
        # HWDGE engines in parallel so they overlap the barrier + warmup.
        def emit_load0(eng, jc, h):
            eng.dma_start(
                a3[0][:, SLOT[jc], P0[h]:P0[h + 1]],
                act_v2[0, jc, :, P0[h]:P0[h + 1]],
            ).then_inc(s_ld0h[h], 16)

        with nc.Block("clears") as blk:

            @blk.sync
            def _(sync):
                for s in [s_ld0h[0], s_mm, s_cast_v, s_st] + s_ld:
                    sync.sem_clear(s)
                for jc in (0, 1):
                    emit_load0(sync, jc, 0)

            @blk.scalar
            def _(scalar):
                for s in (s_gt, s_ld0h[1], s_cast_s, s_stg):
                    scalar.sem_clear(s)
                scalar.dma_start(gt_sb.rearrange(
                    "p (jc c) -> p jc c", jc=NCHUNK, c=C // 2), gt_v,
                ).then_inc(s_gt, 16)
                for jc in (2, 3):
                    emit_load0(scalar, jc, 0)

        with nc.Block("main", no_gpsimd_drain=True) as blk:

            @blk.sync
            def _(sync):
                n_store = 0
                for jc in (0, 1):
                    emit_load0(sync, jc, 1)
                for img in (1, 2, 3):
                    for jc in range(NCHUNK):
                        emit_load(sync, img, jc)
                # stores of zc 0,1 behind vector's casts
                for img in range(IMGS):
                    for lo, hi, cnt in store_pieces(img):
                        sync.wait_ge(s_cast_v, cnt)
                        for zc in (0, 1):
                            sync.dma_start(
                                out_v[img, zc, :, lo:hi],
                                o4[img][:, zc, lo:hi],
                            ).then_inc(s_st, 16)
                            n_store += 1
                sync.wait_ge(s_st, 16 * n_store)

            @blk.gpsimd
            def _(gpsimd):
                n_store = 0
                for img in range(IMGS):
                    for lo, hi, cnt in store_pieces(img):
                        gpsimd.wait_ge(s_cast_s, cnt)
                        for zc in (2, 3):
                            gpsimd.dma_start(
                                out_v[img, zc, :, lo:hi],
                                o4[img][:, zc, lo:hi],
                            ).then_inc(s_stg, 16)
                            n_store += 1
                gpsimd.wait_ge(s_stg, 16 * n_store)

            @blk.tensor
            def _(tensor):
                # HAM warmup on garbage sbuf while gt/act loads land
                # (o_sb[0] is not written until the first casts, which wait
                # on real matmuls; psum target is overwritten start=True)
                for _ in range(6):
                    tensor.matmul(ps_s[:, 512:512 + PT],
                                  o_sb[0][:, :P], o_sb[0][:, :PT],
                                  start=True, stop=True)
                tensor.wait_ge(s_gt, 16)
                for img in range(IMGS):
                    for cb in range(NPT):
                        gidx = img * NPT + cb
                        if img == 0:
                            if cb in (0, 4):
                                tensor.wait_ge(s_ld0h[0 if cb == 0 else 1],
                                               64)
                        elif cb == 0:
                            tensor.wait_ge(s_ld[img], 64)
                        for zc in range(NCHUNK):
                            if gidx >= 2:
                                sem = s_cast_v if zc < 2 else s_cast_s
                                tensor.wait_ge(sem, gidx - 1)
                            sl0, sl1, st = RHS_SL[zc]
                            tensor.matmul(
                                mm_out(zc, gidx % 2),
                                GT_W[zc],
                                a3[img][:, sl0:sl1:st,
                                        cb * PT:(cb + 1) * PT],
                                start=True, stop=True,
                                perf_mode=mybir.MatmulPerfMode.DoubleRow,
                            ).then_inc(s_mm)

            @blk.vector
            def _(vector):
                for gidx in range(GROUPS):
                    img, cb = divmod(gidx, NPT)
                    vector.wait_ge(s_mm, gidx * NCHUNK + 2)
                    vector.tensor_copy(
                        o4[img][:, 0:2, cb * PT:(cb + 1) * PT],
                        psv4[:, :, gidx % 2, :PT],
                    ).then_inc(s_cast_v)

            @blk.scalar
            def _(scalar):
                for jc in (2, 3):
                    emit_load0(scalar, jc, 1)
                for gidx in range(GROUPS):
                    img, cb = divmod(gidx, NPT)
                    scalar.wait_ge(s_mm, gidx * NCHUNK + 4)
                    scalar.copy(
                        o4[img][:, 2:4, cb * PT:(cb + 1) * PT],
                        pss4[:, :, gidx % 2, :PT],
                    ).then_inc(s_cast_s)

    nc.compile()
    return nc


def _make_g(inhib_kernel: np.ndarray) -> np.ndarray:
    k = np.asarray(inhib_kernel, dtype=np.float64)
    return np.real(np.fft.ifft(1.0 / np.fft.fft(k)))


def _make_gt(inhib_kernel: np.ndarray) -> np.ndarray:
    """Weights W[j, r] = SCALE * h[(r + S_ROT - j) mod C] in fp8;
    h = g minus its dominant tap a0 at offset ROT (added back on host)."""
    g = _make_g(inhib_kernel)
    h = g.copy()
    h[ROT] -= g[ROT]
    idx = (np.arange(C)[None, :] + S_ROT - np.arange(C)[:, None]) % C
    w_full = (SCALE * h[idx]).astype(IO_NP)      # [j, r] full circulant
    # pack only the blocks the kernel reads. DRAM row-chunk order
    # (jc0, jc1, jc3, jc2) with each row-chunk's two kept zc-blocks, so
    # sbuf block b = row_pos*2 + half gives every zc an ASCENDING
    # block pair matching its rhs chunk order (see GT_W in _build_nc)
    order = (0, 1, 3, 2)
    kept = {0: (3, 2), 1: (3, 0), 3: (2, 1), 2: (0, 1)}
    w_c = np.empty((C, C // 2), dtype=IO_NP)
    for pos, jc in enumerate(order):
        for half, zc in enumerate(kept[jc]):
            w_c[pos * 128:(pos + 1) * 128, half * 128:(half + 1) * 128] = \
                w_full[jc * 128:(jc + 1) * 128, zc * 128:(zc + 1) * 128]
    return np.ascontiguousarray(w_c)


def kernel(activations, inhib_kernel):
    acts = np.asarray(activations, dtype=np.float32)
    assert acts.shape == (N, C, H, W), acts.shape
    g = _make_g(np.asarray(inhib_kernel))
    a0 = g[ROT]
    gt_np = _make_gt(np.asarray(inhib_kernel))

    if "nc" not in _CACHE:
        _CACHE["nc"] = _build_nc()
    nc = _CACHE["nc"]

    acts_flat = acts.reshape(N, C, HW)
    acts8 = acts_flat.astype(IO_NP)
    in_maps = [
        {"act": np.ascontiguousarray(acts8[c * IMGS:(c + 1) * IMGS]),
         "gt": gt_np}
        for c in range(N_CORES)
    ]
    res = run_bass_kernel_spmd(nc, in_maps, core_ids=list(range(N_CORES)))
    z = np.concatenate([np.asarray(r["out"]) for r in res.results], axis=0)
    # y[i] = a0 * x[(i-ROT) mod C] + z[(i-S_ROT) mod C] / SCALE
    y = np.roll(z, S_ROT, axis=1).astype(np.float32)
    y *= np.float32(1.0 / SCALE)
    y += np.float32(a0) * np.roll(acts_flat, ROT, axis=1)
    return y.reshape(N, C, H, W)
